# revision 1
# baseline (speedup 1.0000x reference)
"""Trainium2 fused single-pass kernel for DetContrastiveLoss (embedding_lookup).

Reference semantics (buggy original preserved): only the LAST batch element of
spatial_features_2d / gt_boxes is used.  500 box centers map to pixel indices,
the 128-channel feature vector at each pixel is gathered, L2-normalized, and a
500x500 cosine-similarity contrastive loss (log_softmax + label pick) reduces
to a scalar.

Single core, single pass (the loss is tiny; the sharding hint sanctions one
device).  Device flow:
  - SP engine DMAs idx [128,4] i32 and aux [128,512] f32 on its own HWDGE queue.
  - gpsimd: 4 indirect DMAs gather 512 padded pixel rows (512 B each) from the
    pixel-major table [HW, 128] into raw [128pix, 4*128ch] (pixel-major tiles).
  - DVE: per-tile sumsq over channels (free axis) with accum_out; ACT computes
    rs = sqrt(1/T) * ss^-1/2 via Ln -> Exp(scale=-0.5, bias=ln(sqrt(1/T)))
    (Rsqrt activation is blocked for accuracy); DVE scales tiles to bf16.
  - PE transposes the 4 normalized tiles (identity built on device from aux's
    iota row and partition-index column) -> fncm [128ch, 512pix] bf16.
  - PE: 4 bf16 matmuls G_t[128,500] = fncm_tile^T @ fncm[:, :500] (= sim/T,
    temperature folded into the normalization scale).
  - ACT: Exp(G_t) with accum_out -> softmax denominators (no max subtraction:
    |sim/T| <= 10 so exp is comfortably in fp32 range).
  - DVE: label pick per tile in ONE op: (iota == label) * G_t with accum_out.
  - val = (pick - ln(ssum)) * validmask, reduced to [128,1]; PE ones-matmul
    reduces partitions; ACT copies PSUM->SBUF; gpsimd DMAs the scalar out.
Host: loss = -sum * LOSS_SCALE / N.
"""

import contextlib
import ctypes
import os
import sys
import types

import numpy as np
import ml_dtypes

from concourse import bass, mybir
from concourse import bass_utils as _bass_utils
from concourse.bass_utils import run_bass_kernel_spmd

# Problem geometry (hardcoded per spec nn_DetContrastiveLoss_72636486910298).
B, C, H, W = 4, 128, 512, 512
HW = H * W
N = 500
NPAD = 512
NT = NPAD // 128  # 4 row tiles

PC_RANGE = (-59.9, -59.9, -2.0, 59.9, 59.9, 5.9)
TEMPERATURE = 0.1
LOSS_SCALE = 0.01
SQRT_INV_TEMP = float(np.sqrt(1.0 / TEMPERATURE))
LN_SQRT_INV_TEMP = float(0.5 * np.log(1.0 / TEMPERATURE))

F32 = mybir.dt.float32
BF16 = mybir.dt.bfloat16
I32 = mybir.dt.int32

LAST_EXEC_NS = {}
LAST_TRACE_DIRS = {}
_DEBUG = {}


def _install_ntff_hook():
    """Provide antenv.axon_hooks (absent in this image) so bass_utils'
    trace=True path can capture NTFF profiles via the axon PJRT .so."""
    try:
        import antenv.axon_hooks  # noqa: F401
        return
    except ImportError:
        pass
    hook = None
    so_path = "/opt/axon/libaxon_pjrt.so"
    if os.path.exists(so_path):
        lib = ctypes.CDLL(so_path)
        if hasattr(lib, "axon_start_nrt_profile"):
            lib.axon_start_nrt_profile.argtypes = [
                ctypes.POINTER(ctypes.c_int64), ctypes.c_size_t]
            lib.axon_start_nrt_profile.restype = ctypes.c_int64
            lib.axon_stop_nrt_profile.argtypes = [ctypes.c_char_p]
            lib.axon_stop_nrt_profile.restype = ctypes.c_int64

            @contextlib.contextmanager
            def _hook(output_dir, device_ids):
                import jax
                jax.devices()
                if device_ids:
                    ids = (ctypes.c_int64 * len(device_ids))(*device_ids)
                    rc = lib.axon_start_nrt_profile(ids, len(device_ids))
                else:
                    rc = lib.axon_start_nrt_profile(None, 0)
                if rc != 0:
                    raise RuntimeError(f"axon_start_nrt_profile rc={rc}")
                try:
                    yield
                finally:
                    n = lib.axon_stop_nrt_profile(str(output_dir).encode())
                    print(f"profile: {n} file(s) -> {output_dir}", file=sys.stderr)

            hook = _hook
    mod = types.ModuleType("antenv.axon_hooks")
    mod.get_axon_ntff_profile_hook = lambda: hook
    mod.set_axon_ntff_profile_hook = lambda h: None
    sys.modules["antenv.axon_hooks"] = mod


def _run(prog_key, in_maps):
    progs = _progs()
    LAST_EXEC_NS.setdefault(prog_key, None)
    LAST_TRACE_DIRS.setdefault(prog_key, None)
    if os.environ.get("KERNEL_TRACE"):
        _install_ntff_hook()
        _bass_utils.upload_artifacts = lambda tmpdir: "local://" + str(tmpdir)
        import tempfile
        tmpdir = tempfile.mkdtemp(prefix=f"bass_{prog_key}_")
        LAST_TRACE_DIRS[prog_key] = tmpdir
        res = run_bass_kernel_spmd(
            progs[prog_key], in_maps, core_ids=list(range(len(in_maps))),
            trace=True, tmpdir=tmpdir,
        )
    else:
        old = os.environ.get("BASS_NEVER_TRACE")
        os.environ["BASS_NEVER_TRACE"] = "1"
        try:
            res = run_bass_kernel_spmd(
                progs[prog_key], in_maps,
                core_ids=list(range(len(in_maps))))
        finally:
            if old is None:
                os.environ.pop("BASS_NEVER_TRACE", None)
            else:
                os.environ["BASS_NEVER_TRACE"] = old
    LAST_EXEC_NS[prog_key] = res.exec_time_ns
    return res


# aux layout (f32 [128, 512]):
#   cols 0..499   iota j (same every partition)
#   cols 500..503 labels (f32) for row tiles 0..3: aux[p, 500+t] = lab[t*128+p]
#   cols 504..507 valid-row mask per tile: 1.0 iff t*128+p < N
#   col  508      partition index p (for the identity build)
AUXW = 512
IOTA0 = 0
LABC = 500
MSKC = 504
PCOL = 508


def _build(upto="full"):
    nc = bass.Bass("TRN2", target_bir_lowering=False)
    # target_bir_lowering=False skips the prologue semaphore/DMA-state clear;
    # after other NEFFs ran on this core (e.g. the grader's jax reference),
    # stale sem values deadlock/corrupt this program.  Clear manually.
    if os.environ.get("KV_SEMCLEAR", "1") == "1":
        for sem_range in bass.compact_to_ranges(
                [s for s in nc._kernel_sem_range if s not in nc.barrier_sems]):
            nc.gpsimd.dma_reset(sem_range)
            nc.gpsimd.sem_clear(sem_range)
        nc._nrt_pseudo_barrier()
    table = nc.dram_tensor("table", [HW, C], F32, kind="ExternalInput")
    idx = nc.dram_tensor("idx", [128, NT], I32, kind="ExternalInput")
    aux = nc.dram_tensor("aux", [128, AUXW], F32, kind="ExternalInput")
    if upto == "fnpm":
        out = nc.dram_tensor("res", [128, NPAD], BF16, kind="ExternalOutput")
    else:
        out = nc.dram_tensor("res", [1, 1], F32, kind="ExternalOutput")

    AF = mybir.ActivationFunctionType
    OP = mybir.AluOpType

    from contextlib import ExitStack
    with ExitStack() as ctx:
        idx_sb = ctx.enter_context(nc.sbuf_tensor([128, NT], I32))
        aux_sb = ctx.enter_context(nc.sbuf_tensor([128, AUXW], F32))
        raw = ctx.enter_context(nc.sbuf_tensor([128, NPAD], F32))
        sq = ctx.enter_context(nc.sbuf_tensor([128, NPAD], F32))
        ss = ctx.enter_context(nc.sbuf_tensor([128, NT], F32))
        lnss = ctx.enter_context(nc.sbuf_tensor([128, NT], F32))
        rs = ctx.enter_context(nc.sbuf_tensor([128, NT], F32))
        fnpm = ctx.enter_context(nc.sbuf_tensor([128, NPAD], BF16))
        ident = ctx.enter_context(nc.sbuf_tensor([128, 128], BF16))
        fncm = ctx.enter_context(nc.sbuf_tensor([128, NPAD], BF16))
        escr = ctx.enter_context(nc.sbuf_tensor([128, N], BF16))
        mscr = ctx.enter_context(nc.sbuf_tensor([128, N], F32))
        ssum = ctx.enter_context(nc.sbuf_tensor([128, NT], F32))
        lnS = ctx.enter_context(nc.sbuf_tensor([128, NT], F32))
        pick = ctx.enter_context(nc.sbuf_tensor([128, NT], F32))
        val4 = ctx.enter_context(nc.sbuf_tensor([128, NT], F32))
        vmask = ctx.enter_context(nc.sbuf_tensor([128, NT], F32))
        val = ctx.enter_context(nc.sbuf_tensor([128, 1], F32))
        ones = ctx.enter_context(nc.sbuf_tensor([128, 1], F32))
        res_sb = ctx.enter_context(nc.sbuf_tensor([1, 1], F32))
        dscr = ctx.enter_context(nc.sbuf_tensor([1, 1], F32))
        biasc = ctx.enter_context(nc.sbuf_tensor([128, 1], F32))

        ptr = ctx.enter_context(nc.psum_tensor("ptr", [128, NPAD], F32))
        G = [ctx.enter_context(nc.psum_tensor(f"G{t}", [128, N], F32))
             for t in range(NT)]
        tot = ctx.enter_context(nc.psum_tensor([1, 1], F32))

        s_in = ctx.enter_context(nc.semaphore())
        s_aux = ctx.enter_context(nc.semaphore())
        s_gt = [ctx.enter_context(nc.semaphore(f"s_g{t}"))
                for t in range(NT)]
        s_v = ctx.enter_context(nc.semaphore())
        s_a = ctx.enter_context(nc.semaphore())
        s_p = ctx.enter_context(nc.semaphore())
        s_o = ctx.enter_context(nc.semaphore())

        block = ctx.enter_context(nc.Block(no_gpsimd_drain=True))

        # Raw-bass hazard discipline: cross-engine releases ride on a drain()
        # of the producing engine; dependent same-engine steps are separated
        # by drains too (a consumer can read the producer's tail otherwise).

        @block.gpsimd
        def _(g):
            g.dma_start(idx_sb[:], idx[:]).then_inc(s_in, 16)
            if upto != "fnpm":
                g.dma_start(aux_sb[:], aux[:]).then_inc(s_aux, 16)
            g.wait_ge(s_in, 16)
            for t in range(NT):
                g.indirect_dma_start(
                    out=raw[:, t * 128:(t + 1) * 128],
                    out_offset=None,
                    in_=table[:],
                    in_offset=bass.IndirectOffsetOnAxis(
                        ap=idx_sb[:, t:t + 1], axis=0),
                ).then_inc(s_gt[t], 16)
            if upto == "fnpm":
                g.wait_ge(s_v, 6)
                g.dma_start(out[:], fnpm[:]).then_inc(s_o, 16)
            else:
                g.wait_ge(s_a, 7)      # res_sb holds the final scalar
                g.dma_start(out[:], res_sb[:]).then_inc(s_o, 16)
            g.wait_ge(s_o, 16)

        @block.vector
        def _(v):
            v.memset(ones[:], 1.0)
            v.memset(biasc[:], LN_SQRT_INV_TEMP)
            if upto != "fnpm":
                v.wait_ge(s_aux, 16)   # aux landed
                v.tensor_scalar(
                    out=ident[:], in0=aux_sb[:, IOTA0:IOTA0 + 128],
                    scalar1=aux_sb[:, PCOL:PCOL + 1], scalar2=None,
                    op0=OP.is_equal)
                v.tensor_copy(vmask[:], aux_sb[:, MSKC:MSKC + NT])
            v.drain().then_inc(s_v, 1)                         # sv=1: ident
            for t in range(NT):
                v.wait_ge(s_gt[t], 16)
                v.tensor_tensor(
                    out=sq[:, t * 128:(t + 1) * 128],
                    in0=raw[:, t * 128:(t + 1) * 128],
                    in1=raw[:, t * 128:(t + 1) * 128],
                    op=OP.mult)
                v.drain()
                v.reduce_sum(out=ss[:, t:t + 1],
                             in_=sq[:, t * 128:(t + 1) * 128],
                             axis=mybir.AxisListType.X)
            v.drain().then_inc(s_v, 1)                         # sv=2: ss
            v.wait_ge(s_a, 1)          # rs ready (ACT Ln+Exp)
            for t in range(NT):
                v.tensor_scalar(
                    out=fnpm[:, t * 128:(t + 1) * 128],
                    in0=raw[:, t * 128:(t + 1) * 128],
                    scalar1=rs[:, t:t + 1], scalar2=None,
                    op0=OP.mult)
                v.drain().then_inc(s_v, 1)                     # sv=3..6: fnpm t
            if upto == "fnpm":
                return
            for t in range(NT):
                v.wait_ge(s_p, 5 + t)  # G[t] in PSUM (PE drained)
                v.scalar_tensor_tensor(
                    out=mscr[:], in0=aux_sb[:, IOTA0:IOTA0 + N],
                    scalar=aux_sb[:, LABC + t:LABC + t + 1],
                    in1=G[t][:], op0=OP.is_equal, op1=OP.mult,
                    accum_out=pick[:, t:t + 1])
                v.drain()
            v.wait_ge(s_a, 6)          # lnS ready
            v.scalar_tensor_tensor(
                out=val4[:], in0=pick[:], scalar=0.0, in1=lnS[:],
                op0=OP.add, op1=OP.subtract)
            v.drain()
            v.tensor_tensor(out=val4[:], in0=val4[:], in1=vmask[:],
                            op=OP.mult)
            v.drain()
            v.reduce_sum(out=val[:], in_=val4[:], axis=mybir.AxisListType.X)
            v.drain().then_inc(s_v, 1)                         # sv=7: val

        @block.tensor
        def _(te):
            if upto == "fnpm":
                return
            for t in range(NT):
                te.wait_ge(s_v, 3 + t)     # fnpm tile t scaled (+ ident)
                nc.tensor.matmul(
                    ptr[:, t * 128:(t + 1) * 128],
                    lhsT=fnpm[:, t * 128:(t + 1) * 128],
                    rhs=ident[:], start=True, stop=True)
                te.drain().then_inc(s_p, 1)  # sp=1..4: ptr tile t
            te.wait_ge(s_a, 5)        # all fncm copies landed
            for t in range(NT):
                nc.tensor.matmul(
                    G[t][:], lhsT=fncm[:, t * 128:(t + 1) * 128],
                    rhs=fncm[:, 0:N], start=True, stop=True)
                te.drain().then_inc(s_p, 1)  # sp=5..8: G[t]
            te.wait_ge(s_v, 7)        # val ready
            nc.tensor.matmul(tot[:], lhsT=val[:], rhs=ones[:],
                             start=True, stop=True)
            te.drain().then_inc(s_p, 1)      # sp=9: tot

        @block.scalar
        def _(a):
            a.activation(out=dscr[:], in_=nc.const_aps.aps[(F32, 0.0)][0:1, 0:1],
                         func=AF.Exp)
            a.wait_ge(s_v, 2)          # ss complete
            a.activation(out=lnss[:], in_=ss[:], func=AF.Ln)
            a.drain()
            a.activation(out=rs[:], in_=lnss[:], func=AF.Exp,
                         scale=-0.5, bias=biasc[:])
            a.drain().then_inc(s_a, 1)                         # sa=1: rs
            if upto == "fnpm":
                return
            for t in range(NT):
                a.wait_ge(s_p, t + 1)  # ptr tile t transposed
                a.activation(out=fncm[:, t * 128:(t + 1) * 128],
                             in_=ptr[:, t * 128:(t + 1) * 128],
                             func=AF.Copy)
                a.drain().then_inc(s_a, 1)                     # sa=2..5
            for t in range(NT):
                a.wait_ge(s_p, 5 + t)  # G[t] ready
                a.activation(out=escr[:], in_=G[t][:], func=AF.Exp,
                             accum_out=ssum[:, t:t + 1])
                a.drain()
            a.activation(out=lnS[:], in_=ssum[:], func=AF.Ln)
            a.drain().then_inc(s_a, 1)                         # sa=6: lnS
            a.wait_ge(s_p, 9)          # tot in PSUM
            a.activation(out=res_sb[:], in_=tot[:], func=AF.Copy)
            a.drain().then_inc(s_a, 1)                         # sa=7: res_sb
    return nc


def _build_loss():
    """bf16 matmuls from SBUF + Exp accum + stt pick accum + tot matmul."""
    nc = bass.Bass("TRN2", target_bir_lowering=False)
    fni = nc.dram_tensor("fni", [128, NPAD], BF16, kind="ExternalInput")
    aux = nc.dram_tensor("aux", [128, AUXW], F32, kind="ExternalInput")
    res = nc.dram_tensor("res", [1, 1], F32, kind="ExternalOutput")
    AF = mybir.ActivationFunctionType
    OP = mybir.AluOpType
    from contextlib import ExitStack
    with ExitStack() as ctx:
        fncm = ctx.enter_context(nc.sbuf_tensor([128, NPAD], BF16))
        aux_sb = ctx.enter_context(nc.sbuf_tensor([128, AUXW], F32))
        escr = ctx.enter_context(nc.sbuf_tensor([128, N], BF16))
        mscr = ctx.enter_context(nc.sbuf_tensor([128, N], F32))
        ssum = ctx.enter_context(nc.sbuf_tensor([128, NT], F32))
        lnS = ctx.enter_context(nc.sbuf_tensor([128, NT], F32))
        pick = ctx.enter_context(nc.sbuf_tensor([128, NT], F32))
        val4 = ctx.enter_context(nc.sbuf_tensor([128, NT], F32))
        vmask = ctx.enter_context(nc.sbuf_tensor([128, NT], F32))
        val = ctx.enter_context(nc.sbuf_tensor([128, 1], F32))
        ones = ctx.enter_context(nc.sbuf_tensor([128, 1], F32))
        res_sb = ctx.enter_context(nc.sbuf_tensor([1, 1], F32))
        dscr = ctx.enter_context(nc.sbuf_tensor([1, 1], F32))
        G = [ctx.enter_context(nc.psum_tensor(f"G{t}", [128, N], F32))
             for t in range(NT)]
        tot = ctx.enter_context(nc.psum_tensor([1, 1], F32))
        s_in = ctx.enter_context(nc.semaphore())
        s_aux = ctx.enter_context(nc.semaphore())
        s_v = ctx.enter_context(nc.semaphore())
        s_a = ctx.enter_context(nc.semaphore())
        s_p = ctx.enter_context(nc.semaphore())
        s_o = ctx.enter_context(nc.semaphore())
        block = ctx.enter_context(nc.Block(no_gpsimd_drain=True))

        @block.gpsimd
        def _(g):
            g.dma_start(fncm[:], fni[:]).then_inc(s_in, 16)
            g.dma_start(aux_sb[:], aux[:]).then_inc(s_aux, 16)
            g.wait_ge(s_a, 7)
            g.dma_start(res[:], res_sb[:]).then_inc(s_o, 16)
            g.wait_ge(s_o, 16)

        @block.vector
        def _(v):
            v.memset(ones[:], 1.0)
            v.wait_ge(s_aux, 16)
            v.tensor_copy(vmask[:], aux_sb[:, MSKC:MSKC + NT])
            v.drain().then_inc(s_v, 1)
            for t in range(NT):
                v.wait_ge(s_p, 1 + t)
                v.scalar_tensor_tensor(
                    out=mscr[:], in0=aux_sb[:, IOTA0:IOTA0 + N],
                    scalar=aux_sb[:, LABC + t:LABC + t + 1],
                    in1=G[t][:], op0=OP.is_equal, op1=OP.mult,
                    accum_out=pick[:, t:t + 1])
                v.drain()
            v.wait_ge(s_a, 6)
            v.scalar_tensor_tensor(
                out=val4[:], in0=pick[:], scalar=0.0, in1=lnS[:],
                op0=OP.add, op1=OP.subtract)
            v.drain()
            v.tensor_tensor(out=val4[:], in0=val4[:], in1=vmask[:],
                            op=OP.mult)
            v.drain()
            v.reduce_sum(out=val[:], in_=val4[:], axis=mybir.AxisListType.X)
            v.drain().then_inc(s_v, 1)

        @block.tensor
        def _(te):
            te.wait_ge(s_in, 16)
            for t in range(NT):
                nc.tensor.matmul(
                    G[t][:], lhsT=fncm[:, t * 128:(t + 1) * 128],
                    rhs=fncm[:, 0:N], start=True, stop=True)
                te.drain().then_inc(s_p, 1)
            te.wait_ge(s_v, 2)
            nc.tensor.matmul(tot[:], lhsT=val[:], rhs=ones[:],
                             start=True, stop=True)
            te.drain().then_inc(s_p, 1)

        @block.scalar
        def _(a):
            a.activation(out=dscr[:], in_=nc.const_aps.aps[(F32, 0.0)][0:1, 0:1],
                         func=AF.Exp)
            for t in range(NT):
                a.wait_ge(s_p, 1 + t)
                a.activation(out=escr[:], in_=G[t][:], func=AF.Exp,
                             accum_out=ssum[:, t:t + 1])
                a.drain()
            a.activation(out=lnS[:], in_=ssum[:], func=AF.Ln)
            a.drain().then_inc(s_a, 6)
            a.wait_ge(s_p, 5)
            a.activation(out=res_sb[:], in_=tot[:], func=AF.Copy)
            a.drain().then_inc(s_a, 1)
    return nc, ["res"]


_PROGS = {}


def _progs():
    if not _PROGS:
        _PROGS["fnpmA"] = _build("fnpm")
        _PROGS["lossB"] = _build_loss()[0]
    return _PROGS


def _pixel_indices(gt_boxes: np.ndarray) -> np.ndarray:
    """Exact fp32 replication of the reference pixel-index math (last batch)."""
    boxes = np.asarray(gt_boxes)[B - 1].astype(np.float32, copy=False)
    x = boxes[:, 0].astype(np.float32)
    y = boxes[:, 1].astype(np.float32)
    span_w = PC_RANGE[3] - PC_RANGE[0]
    span_h = PC_RANGE[4] - PC_RANGE[1]
    cx = (x - np.float32(PC_RANGE[0])) / np.float32(span_w) * np.float32(W)
    cy = (y - np.float32(PC_RANGE[1])) / np.float32(span_h) * np.float32(H)
    cx = np.clip(cx.astype(np.int32), 0, W - 1)
    cy = np.clip(cy.astype(np.int32), 0, H - 1)
    return (cy.astype(np.int64) * W + cx.astype(np.int64)).astype(np.int32)


def kernel(spatial_features_2d, gt_boxes, static_labels, dynamic_labels,
           num_static=None, **_unused):
    _progs()
    sf = np.asarray(spatial_features_2d)
    pix = _pixel_indices(gt_boxes)

    pix_pad = np.zeros(NPAD, dtype=np.int32)
    pix_pad[:N] = pix
    idx = np.ascontiguousarray(pix_pad.reshape(NT, 128).T)
    table = np.ascontiguousarray(sf[B - 1].reshape(C, HW).T, dtype=np.float32)

    labels = np.concatenate(
        [np.asarray(static_labels), np.asarray(dynamic_labels)], axis=0
    ).astype(np.int64)
    lab_pad = np.zeros(NPAD, dtype=np.float32)
    lab_pad[:N] = labels.astype(np.float32)
    msk_pad = np.zeros(NPAD, dtype=np.float32)
    msk_pad[:N] = 1.0
    aux_np = np.zeros((128, AUXW), dtype=np.float32)
    aux_np[:, IOTA0:IOTA0 + N] = np.arange(N, dtype=np.float32)[None, :]
    aux_np[:, LABC:LABC + NT] = lab_pad.reshape(NT, 128).T
    aux_np[:, MSKC:MSKC + NT] = msk_pad.reshape(NT, 128).T
    aux_np[:, PCOL] = np.arange(128, dtype=np.float32)

    rA = _run("fnpmA", [{"table": table, "idx": idx, "aux": aux_np}])
    fnpm = np.asarray(rA.results[0]["res"])  # [128, NPAD] bf16 pixel-major
    fncm = np.concatenate(
        [np.ascontiguousarray(fnpm[:, t * 128:(t + 1) * 128].T)
         for t in range(NT)], axis=1).astype(ml_dtypes.bfloat16)

    rB = _run("lossB", [{"fni": fncm, "aux": aux_np}])
    total = float(np.asarray(rB.results[0]["res"])[0, 0])
    loss = np.float32(-total * LOSS_SCALE / N)
    return np.array(loss, dtype=np.float32)

